# revision 25
# baseline (speedup 1.0000x reference)
"""Longformer (chunked sliding-window) self-attention on 8 TRN2 NeuronCores.

Sharding: sequence-parallel. B=2, L=4096 -> 8 blocks of 1024 query tokens
(4 blocks per batch element), one block per core. Each core also receives a
512-token K/V halo (the previous chunk), so no cross-core communication is
needed. The first block of each batch gets a zero halo; halo keys are made
invalid not by an additive mask but by a per-key validity column in V, which
drops them from both softmax numerator and denominator exactly like the
reference's -1e9 masking.

On-chip layout choices (per core):
  - x is passed pre-transposed (xT [D, NKV], bf16); weights pre-transposed
    (W.T, [din, dout], bf16).
  - q, k are produced transposed ([d, tok], bf16); v natural ([tok, d], bf16)
    with a validity column appended per head (1 valid / 0 halo).
  - scores are computed transposed (k_tok on psum partitions); two heads of a
    pair share one [128, 1024] 2-bank PSUM tile so one ScalarE exp covers both.
  - A*V runs in NATURAL orientation: lhsT = p^T tile [k,128q] (stationary),
    rhs = v tile [k, 65] (64 dims + validity column) -> psum [128q, 65]
    accumulated over the 8 k-tiles. This halves the PE cost vs the transposed
    formulation (moving dim 65 instead of 512 per head) and lands the softmax
    denominator in column 64 of the same psum tile.
  - softmax division fuses into the psum drain: per-partition reciprocal of
    column 64, then one broadcast multiply per head writes normalized ctx
    (natural [tok, d], bf16) to SBUF. No selection matmuls, no denominator
    gather DMAs.
  - ctx is transposed for the output projection by XBAR DMA-transpose
    (SBUF->SBUF, one [128q x 128d] instruction per (q-tile, head pair),
    emitted right after that pair's drain), costing no PE or DVE cycles.
  - Scheduling: the attention exp stream is the per-chunk pacing item, so all
    non-prelude projection work (q stripe 1, k stripe 2, v tiles 8-11, the
    n=1 half of v, and the chunk-0 output projection) is chopped into
    single-matmul thunks and dripped into the attention loop between k-tile
    iterations, keeping the PE dense (which also keeps it at full p-state).
"""

from collections import deque

import numpy as np

B, L, D = 2, 4096, 1024
H, DH, W = 16, 64, 512
NCORES = 8
BLK = L // 4          # 1024 query tokens per core
NKV = BLK + W         # 1536 kv tokens (halo + own)
CHUNKS = BLK // W     # 2 chunks per core
KT = (2 * W) // 128   # 8 k-token tiles of 128 per chunk window

_CACHE = {}


def _build():
    import concourse.bacc as bacc
    import concourse.mybir as mybir
    import concourse.tile as tile

    f32 = mybir.dt.float32
    bf16 = mybir.dt.bfloat16
    AF = mybir.ActivationFunctionType

    nc = bacc.Bacc("TRN2", target_bir_lowering=False, debug=False,
                   num_devices=NCORES)

    xT = nc.dram_tensor("xT", [D, NKV], bf16, kind="ExternalInput").ap()
    wqT = nc.dram_tensor("wqT", [D, D], bf16, kind="ExternalInput").ap()
    wkT = nc.dram_tensor("wkT", [D, D], bf16, kind="ExternalInput").ap()
    wvT = nc.dram_tensor("wvT", [D, D], bf16, kind="ExternalInput").ap()
    woT = nc.dram_tensor("woT", [D, D], bf16, kind="ExternalInput").ap()
    bqr = nc.dram_tensor("bqr", [128, 8], f32, kind="ExternalInput").ap()
    bkr = nc.dram_tensor("bkr", [128, 8], f32, kind="ExternalInput").ap()
    bvrep = nc.dram_tensor("bvrep", [128, D], bf16, kind="ExternalInput").ap()
    borep = nc.dram_tensor("borep", [128, D], bf16, kind="ExternalInput").ap()
    vones = nc.dram_tensor("vones", [128, 12], f32, kind="ExternalInput").ap()
    out = nc.dram_tensor("out", [BLK, D], f32, kind="ExternalOutput").ap()

    xT_r = xT.rearrange("(ko p) t -> p ko t", p=128)     # [128, 8, 1536]
    wq_r = wqT.rearrange("(ko p) d -> p ko d", p=128)    # [128, 8, 1024]
    wk_r = wkT.rearrange("(ko p) d -> p ko d", p=128)
    wv_r = wvT.rearrange("(ko p) d -> p ko d", p=128)
    wo_r = woT.rearrange("(ko p) d -> p ko d", p=128)
    out_r = out.rearrange("(to p) d -> p to d", p=128)   # [128, 8, 1024]

    with tile.TileContext(nc) as tc:
        with (
            tc.tile_pool(name="const", bufs=1) as constp,
            tc.tile_pool(name="xw", bufs=1) as xwp,
            tc.tile_pool(name="wts", bufs=3) as wp,
            tc.tile_pool(name="acts", bufs=1) as actp,
            tc.tile_pool(name="ptiles", bufs=16) as pp,
            tc.tile_pool(name="normp", bufs=4) as normp,
            tc.tile_pool(name="outs", bufs=4) as op,
            tc.tile_pool(name="psA", bufs=2, space="PSUM") as psA,
            tc.tile_pool(name="psS", bufs=2, space="PSUM") as psS,
            tc.tile_pool(name="psV", bufs=1, space="PSUM") as psV,
        ):
            # ---- inputs, ordered by first need ----
            bq_sb = constp.tile([128, 8], f32)
            bk_sb = constp.tile([128, 8], f32)
            vones_sb = constp.tile([128, 12], f32)

            x_sb = xwp.tile([128, 8, NKV], bf16)         # 24 KB/part
            wq_sb = wp.tile([128, 8, D], bf16, tag="w")
            wk_sb = wp.tile([128, 8, D], bf16, tag="w")
            wv_sb = wp.tile([128, 8, D], bf16, tag="w")
            # three parallel load streams ordered by first need: the
            # critical wq/wk + x stripe 1 split across both HWDGE queues,
            # later x stripes on the GPSIMD software DGE
            nc.scalar.dma_start(bq_sb[:], bqr[:])
            for ko in range(8):
                nc.sync.dma_start(wq_sb[:, ko], wq_r[:, ko])
                nc.gpsimd.dma_start(x_sb[:, ko, 512:1024],
                                    xT_r[:, ko, 512:1024])
            nc.scalar.dma_start(bk_sb[:], bkr[:])
            nc.scalar.dma_start(vones_sb[:], vones[:])
            for ko in range(8):
                nc.sync.dma_start(wk_sb[:, ko], wk_r[:, ko])
                nc.gpsimd.dma_start(x_sb[:, ko, 0:512], xT_r[:, ko, 0:512])
            bo_sb = constp.tile([128, D], bf16)
            nc.scalar.dma_start(bo_sb[:], borep[:])
            for ko in range(8):
                nc.sync.dma_start(wv_sb[:, ko], wv_r[:, ko])
                nc.gpsimd.dma_start(x_sb[:, ko, 1024:1536],
                                    xT_r[:, ko, 1024:1536])
            x_mm = x_sb[:]

            bv_sb = constp.tile([128, D], bf16)
            nc.scalar.dma_start(bv_sb[:], bvrep[:])
            wo_box = {}

            # ---- persistent activations ----
            q_sb = actp.tile([128, 8, BLK], bf16, tag="q")    # q^T [d, tok]
            k_sb = actp.tile([128, 8, NKV], bf16, tag="k")    # k^T [d, tok]
            v_sb = actp.tile([128, 12, H * (DH + 1)], bf16, tag="v")
            # ctx natural [q-part, j, head, dh], one chunk at a time
            ctxn_sb = actp.tile([128, 4, H, DH], bf16, tag="ctxn")
            ctxT_sb = actp.tile([128, 8, BLK], bf16, tag="ctxT")  # ctx^T

            v_ones = v_sb[:].rearrange("p t (h e) -> p t h e", e=DH + 1)

            # ---- thunk-granular work queue dripped into attention ----
            work = deque()

            def drip(n):
                for _ in range(n):
                    if work:
                        work.popleft()()

            def g_proj_qk(w_mm, dst, bias, xn, dn, m):
                """8 matmul thunks for one m-tile of a q^T/k^T stripe; the
                last thunk also adds the bias (per-partition scalar)."""
                box = {}

                def mk(ko):
                    def f():
                        if ko == 0:
                            box["ps"] = psA.tile([128, 512], f32,
                                                 name="ps", tag="ps")
                        nc.tensor.matmul(
                            box["ps"][:],
                            w_mm[:, ko, m * 128:(m + 1) * 128],
                            x_mm[:, ko, xn * 512:(xn + 1) * 512],
                            start=(ko == 0), stop=(ko == 7),
                        )
                        if ko == 7:
                            nc.vector.tensor_scalar_add(
                                dst[:, m, dn * 512:dn * 512 + 512],
                                box["ps"][:], bias[:, m:m + 1],
                            )
                    return f
                return [mk(ko) for ko in range(8)]

            def g_proj_v(t, n):
                """8 matmul thunks for one [128-token x 8-head] v tile; the
                last adds bias and zeroes halo rows."""
                box = {}

                def mk(ko):
                    def f():
                        if ko == 0:
                            box["ps"] = psA.tile([128, 512], f32,
                                                 name="ps", tag="ps")
                        nc.tensor.matmul(
                            box["ps"][:],
                            x_mm[:, ko, t * 128:(t + 1) * 128],
                            wv_sb[:, ko, n * 512:(n + 1) * 512],
                            start=(ko == 0), stop=(ko == 7),
                        )
                        if ko == 7:
                            dst = v_ones[:, t, n * 8:(n + 1) * 8, :DH]
                            nc.vector.tensor_add(
                                dst,
                                box["ps"][:].rearrange("p (h e) -> p h e",
                                                       e=DH),
                                bv_sb[:, n * 512:(n + 1) * 512]
                                .rearrange("p (h e) -> p h e", e=DH),
                            )
                            if t < 4:
                                nc.vector.tensor_scalar_mul(
                                    dst, dst, vones_sb[:, t:t + 1])
                    return f
                return [mk(ko) for ko in range(8)]

            def g_out_proj(to, n):
                """8 matmul thunks for one [128-token x 512] out tile; the
                last adds bias and stores."""
                box = {}

                def mk(ko):
                    def f():
                        if ko == 0:
                            box["ps"] = psA.tile([128, 512], f32,
                                                 name="ps", tag="ps")
                        nc.tensor.matmul(
                            box["ps"][:],
                            ctxT_sb[:, ko, to * 128:(to + 1) * 128],
                            wo_box["t"][:, ko, n * 512:(n + 1) * 512],
                            start=(ko == 0), stop=(ko == 7),
                        )
                        if ko == 7:
                            # bias + store in halves so the final add/store
                            # chain after the last matmul pipelines
                            o_t = op.tile([128, 512], f32, tag="o")
                            for hh in range(2):
                                hsl = slice(hh * 256, (hh + 1) * 256)
                                nc.vector.tensor_add(
                                    o_t[:, hsl], box["ps"][:, hsl],
                                    bo_sb[:, n * 512 + hh * 256:
                                          n * 512 + (hh + 1) * 256])
                                # SWDGE store: keeps the exp-busy Act
                                # sequencer and the transpose-busy SP
                                # sequencer free
                                nc.gpsimd.dma_start(
                                    out_r[:, to, n * 512 + hh * 256:
                                          n * 512 + (hh + 1) * 256],
                                    o_t[:, hsl])
                    return f
                return [mk(ko) for ko in range(8)]

            def attn_scores(c, u, drips, avq):
                """Chunk c, head pair (2u, 2u+1): per k-tile, the two heads'
                transposed scores share one 2-bank PSUM tile and one exp.
                Each slot also runs one deferred A*V group of the PREVIOUS
                pair (from avq) plus drips[i] fill thunks, keeping the PE
                dense through the exp-paced stretch. Returns the pair's 8
                exp'd p tiles."""
                qsl = slice(c * 512, (c + 1) * 512)
                pts = []
                for i in range(KT):
                    ksl = slice(c * 512 + i * 128, c * 512 + (i + 1) * 128)
                    sps = psS.tile([128, 1024], f32, name="sps")
                    nc.tensor.matmul(sps[:, 0:512],
                                     k_sb[0:64, u, ksl], q_sb[0:64, u, qsl],
                                     start=True, stop=True)
                    nc.tensor.matmul(sps[:, 512:1024],
                                     k_sb[64:128, u, ksl], q_sb[64:128, u, qsl],
                                     start=True, stop=True)
                    p_t = pp.tile([128, 1024], bf16, tag="p")
                    nc.scalar.activation(p_t[:], sps[:], AF.Exp, scale=0.125)
                    pts.append(p_t)
                    if avq:
                        avq.popleft()()
                    drip(drips[i])
                return pts

            def make_av_groups(c, u, pts):
                """A*V for pair (c, u), natural orientation, as 8 thunks (one
                per (q-subtile j, head)). Each thunk's 8 accumulating matmuls
                are contiguous, as hardware PSUM accumulation requires groups
                in one bank not to interleave; groups in different banks (e
                vs o, scores, fills) may. Consumed one per slot during the
                NEXT pair's attn_scores."""
                hs_e = (2 * u) * (DH + 1)
                hs_o = (2 * u + 1) * (DH + 1)
                box = {}

                def g(j, par):
                    def f():
                        if j == 0 and par == 0:
                            box["e"] = psV.tile([128, 4, DH + 1], f32,
                                                name="av_e", tag="av_e")
                            box["o"] = psV.tile([128, 4, DH + 1], f32,
                                                name="av_o", tag="av_o")
                        av = box["e"] if par == 0 else box["o"]
                        hs = hs_e if par == 0 else hs_o
                        off = 512 * par
                        for i in range(KT):
                            nc.tensor.matmul(
                                av[:, j],
                                pts[i][:, off + j * 128:off + (j + 1) * 128],
                                v_sb[:, 4 * c + i, hs:hs + DH + 1],
                                start=(i == 0), stop=(i == KT - 1),
                            )
                    return f
                return deque(g(j, par) for j in range(4) for par in (0, 1)), box

            def drain_pair(c, u, box, fine=False):
                """Normalize by the denominator column while copying psum ->
                sbuf (natural ctx, bf16), then transpose this pair's 128
                d-columns per q-tile to ctx^T via XBAR DMA
                (out[p, t] = in[t, p]). fine=True drains q-tile 0 first and
                issues its transpose immediately (tail-latency path)."""
                jsets = [(slice(0, 1), [0]), (slice(1, 4), [1, 2, 3])] \
                    if fine else [(slice(0, 4), [0, 1, 2, 3])]
                for jsl, js in jsets:
                    nj = len(js)
                    for par, key in ((0, "e"), (1, "o")):
                        h = 2 * u + par
                        av = box[key]
                        rec = normp.tile([128, nj, 1], f32, tag="rec")
                        nc.vector.reciprocal(rec[:], av[:, jsl, DH:DH + 1])
                        nc.vector.tensor_mul(
                            ctxn_sb[:, jsl, h, :],
                            av[:, jsl, 0:DH],
                            rec[:].to_broadcast([128, nj, DH]),
                        )
                    for j in js:
                        nc.sync.dma_start_transpose(
                            ctxT_sb[:, u,
                                    c * 512 + j * 128:c * 512 + (j + 1) * 128],
                            ctxn_sb[:, j, 2 * u:2 * u + 2, :]
                            .rearrange("p h e -> p (h e)"),
                        )

            # ---- prelude: chunk-0 attention deps + q stripe 1 (so wq is
            # free for wo's pool slot before the output projection) ----
            for m in range(8):                    # q stripe 0 (chunk-0 qs)
                for f in g_proj_qk(wq_sb[:], q_sb, bq_sb, 1, 0, m):
                    f()
            for m in range(8):                    # k stripe 1 (reuses x s1)
                for f in g_proj_qk(wk_sb[:], k_sb, bk_sb, 1, 1, m):
                    f()
            # validity column per head: 1 for valid keys, 0 for halo keys
            nc.vector.tensor_copy(
                v_ones[:, :, :, DH],
                vones_sb[:, :, None].to_broadcast([128, 12, H]),
            )
            for m in range(8):                    # k stripe 0 (halo)
                for f in g_proj_qk(wk_sb[:], k_sb, bk_sb, 0, 0, m):
                    f()
            for t in range(8):                    # v heads 0..7, kv tiles 0..7
                for f in g_proj_v(t, 0):
                    f()

            # ---- chunk-0 fill queue: things chunk 1 needs ----
            for t in range(8):
                work.extend(g_proj_v(t, 1))       # v heads 8..15, tiles 0..7
            for t in range(8, 12):
                work.extend(g_proj_v(t, 0))       # v heads 0..7, tiles 8..11
            work.extend(g_proj_qk(wk_sb[:], k_sb, bk_sb, 2, 2, 0))  # k2 m0
            work.extend(g_proj_qk(wk_sb[:], k_sb, bk_sb, 2, 2, 1))  # k2 m1
            work.extend(g_proj_qk(wq_sb[:], q_sb, bq_sb, 2, 1, 0))  # q1 m0
            # ---- software-pipelined attention: pair u's A*V groups run
            # inside pair u+1's score/exp slots ----
            pend = None                           # (avq, box, c, u)
            for c in range(CHUNKS):
                if c == 1:
                    # chunk-1 fill queue: q stripe 1 first (pair u's scores
                    # need q1 m_u; m0 came from the chunk-0 queue), then the
                    # rest of k2 / v n=1 tiles 8-11 / chunk-0 out projection
                    work.extend(g_proj_qk(wq_sb[:], q_sb, bq_sb, 2, 1, 1))
                    for m in range(2, 5):
                        work.extend(g_proj_qk(wq_sb[:], q_sb, bq_sb, 2, 1, m))
                        work.extend(g_proj_qk(wk_sb[:], k_sb, bk_sb, 2, 2, m))
                    for m in range(5, 8):
                        work.extend(g_proj_qk(wq_sb[:], q_sb, bq_sb, 2, 1, m))
                    for m in range(5, 8):
                        work.extend(g_proj_qk(wk_sb[:], k_sb, bk_sb, 2, 2, m))
                    for t in range(8, 12):
                        work.extend(g_proj_v(t, 1))
                    # wo reuses wq's pool slot (free once q1 m7 runs); the
                    # loads go via the software DGE so they cannot head-of-
                    # line-block the transpose stream on sync
                    wo_sb = wp.tile([128, 8, D], bf16, tag="w")
                    for ko in range(8):
                        nc.gpsimd.dma_start(wo_sb[:, ko], wo_r[:, ko])
                    wo_box["t"] = wo_sb
                    for to in range(4):
                        for n in range(2):
                            work.extend(g_out_proj(to, n))
                for u in range(8):
                    avq = pend[0] if pend else deque()
                    if c == 0:
                        drips = [3] * 8 if u == 0 else [2] * 8
                    else:
                        drips = [3] * 8 if u < 6 else [2] * 8
                    pts = attn_scores(c, u, drips, avq)
                    if pend:
                        drain_pair(pend[2], pend[3], pend[1])
                    avq, box = make_av_groups(c, u, pts)
                    pend = (avq, box, c, u)

            # ---- tail: last pair's A*V + drain, remaining fills bridge the
            # drain/transpose latency, then the chunk-1 output projection ----
            for f in pend[0]:
                f()
                drip(4)
            drain_pair(pend[2], pend[3], pend[1], fine=True)
            while work:
                work.popleft()()
            for to in range(4, 8):
                for n in range(2):
                    for f in g_out_proj(to, n):
                        f()

    nc.compile()
    return nc


def _host_prep(x, Wq, bq, Wk, bk, Wv, bv, Wo, bo):
    import ml_dtypes

    bf = ml_dtypes.bfloat16
    x = np.ascontiguousarray(np.asarray(x, dtype=np.float32))
    mats = {
        "wqT": np.ascontiguousarray(np.asarray(Wq, np.float32).T.astype(bf)),
        "wkT": np.ascontiguousarray(np.asarray(Wk, np.float32).T.astype(bf)),
        "wvT": np.ascontiguousarray(np.asarray(Wv, np.float32).T.astype(bf)),
        "woT": np.ascontiguousarray(np.asarray(Wo, np.float32).T.astype(bf)),
        "bqr": np.ascontiguousarray(
            np.asarray(bq, np.float32).reshape(8, 128).T),
        "bkr": np.ascontiguousarray(
            np.asarray(bk, np.float32).reshape(8, 128).T),
        "bvrep": np.ascontiguousarray(
            np.tile(np.asarray(bv, np.float32)[None, :], (128, 1)).astype(bf)),
        "borep": np.ascontiguousarray(
            np.tile(np.asarray(bo, np.float32)[None, :], (128, 1)).astype(bf)),
    }

    in_maps = []
    for core in range(NCORES):
        b, j = core // 4, core % 4
        start = j * BLK
        xkv = np.zeros((NKV, D), np.float32)
        lo = start - W
        if lo < 0:
            xkv[W:] = x[b, start:start + BLK]
        else:
            xkv[:] = x[b, lo:start + BLK]
        vo = np.ones((128, 12), np.float32)
        if j == 0:
            vo[:, 0:4] = 0.0         # halo keys (tokens 0..511) are invalid
        im = dict(mats)
        im["xT"] = np.ascontiguousarray(xkv.T.astype(bf))
        im["vones"] = vo
        in_maps.append(im)
    return in_maps


def kernel(x, Wq, bq, Wk, bk, Wv, bv, Wo, bo):
    from concourse.bass_utils import run_bass_kernel_spmd

    if "nc" not in _CACHE:
        _CACHE["nc"] = _build()
    nc = _CACHE["nc"]

    in_maps = _host_prep(x, Wq, bq, Wk, bk, Wv, bv, Wo, bo)
    res = run_bass_kernel_spmd(nc, in_maps, list(range(NCORES)))

    out = np.empty((B, L, D), np.float32)
    for core in range(NCORES):
        b, j = core // 4, core % 4
        out[b, j * BLK:(j + 1) * BLK] = res.results[core]["out"]
    return out


# revision 26
# speedup vs baseline: 1.0285x; 1.0285x over previous
"""Longformer (chunked sliding-window) self-attention on 8 TRN2 NeuronCores.

Sharding: sequence-parallel. B=2, L=4096 -> 8 blocks of 1024 query tokens
(4 blocks per batch element), one block per core. Each core also receives a
512-token K/V halo (the previous chunk), so no cross-core communication is
needed. The first block of each batch gets a zero halo; halo keys are made
invalid not by an additive mask but by a per-key validity column in V, which
drops them from both softmax numerator and denominator exactly like the
reference's -1e9 masking.

On-chip layout choices (per core):
  - x is passed pre-transposed (xT [D, NKV], bf16); weights pre-transposed
    (W.T, [din, dout], bf16).
  - q, k are produced transposed ([d, tok], bf16); v natural ([tok, d], bf16)
    with a validity column appended per head (1 valid / 0 halo).
  - scores are computed transposed (k_tok on psum partitions); two heads of a
    pair share one [128, 1024] 2-bank PSUM tile so one ScalarE exp covers both.
  - A*V runs in NATURAL orientation: lhsT = p^T tile [k,128q] (stationary),
    rhs = v tile [k, 65] (64 dims + validity column) -> psum [128q, 65]
    accumulated over the 8 k-tiles. This halves the PE cost vs the transposed
    formulation (moving dim 65 instead of 512 per head) and lands the softmax
    denominator in column 64 of the same psum tile.
  - softmax division fuses into the psum drain: per-partition reciprocal of
    column 64, then one broadcast multiply per head writes normalized ctx
    (natural [tok, d], bf16) to SBUF. No selection matmuls, no denominator
    gather DMAs.
  - ctx is transposed for the output projection by XBAR DMA-transpose
    (SBUF->SBUF, one [128q x 128d] instruction per (q-tile, head pair),
    emitted right after that pair's drain), costing no PE or DVE cycles.
  - Scheduling: the attention exp stream is the per-chunk pacing item, so all
    non-prelude projection work (q stripe 1, k stripe 2, v tiles 8-11, the
    n=1 half of v, and the chunk-0 output projection) is chopped into
    single-matmul thunks and dripped into the attention loop between k-tile
    iterations, keeping the PE dense (which also keeps it at full p-state).
"""

from collections import deque

import numpy as np

B, L, D = 2, 4096, 1024
H, DH, W = 16, 64, 512
NCORES = 8
BLK = L // 4          # 1024 query tokens per core
NKV = BLK + W         # 1536 kv tokens (halo + own)
CHUNKS = BLK // W     # 2 chunks per core
KT = (2 * W) // 128   # 8 k-token tiles of 128 per chunk window

_CACHE = {}


def _build():
    import concourse.bacc as bacc
    import concourse.mybir as mybir
    import concourse.tile as tile

    f32 = mybir.dt.float32
    bf16 = mybir.dt.bfloat16
    AF = mybir.ActivationFunctionType

    nc = bacc.Bacc("TRN2", target_bir_lowering=False, debug=False,
                   num_devices=NCORES)

    xT = nc.dram_tensor("xT", [D, NKV], bf16, kind="ExternalInput").ap()
    wqT = nc.dram_tensor("wqT", [D, D], bf16, kind="ExternalInput").ap()
    wkT = nc.dram_tensor("wkT", [D, D], bf16, kind="ExternalInput").ap()
    wvT = nc.dram_tensor("wvT", [D, D], bf16, kind="ExternalInput").ap()
    woT = nc.dram_tensor("woT", [D, D], bf16, kind="ExternalInput").ap()
    bqr = nc.dram_tensor("bqr", [128, 8], f32, kind="ExternalInput").ap()
    bkr = nc.dram_tensor("bkr", [128, 8], f32, kind="ExternalInput").ap()
    bvrep = nc.dram_tensor("bvrep", [128, D], bf16, kind="ExternalInput").ap()
    borep = nc.dram_tensor("borep", [128, D], bf16, kind="ExternalInput").ap()
    vones = nc.dram_tensor("vones", [128, 12], f32, kind="ExternalInput").ap()
    out = nc.dram_tensor("out", [BLK, D], f32, kind="ExternalOutput").ap()

    xT_r = xT.rearrange("(ko p) t -> p ko t", p=128)     # [128, 8, 1536]
    wq_r = wqT.rearrange("(ko p) d -> p ko d", p=128)    # [128, 8, 1024]
    wk_r = wkT.rearrange("(ko p) d -> p ko d", p=128)
    wv_r = wvT.rearrange("(ko p) d -> p ko d", p=128)
    wo_r = woT.rearrange("(ko p) d -> p ko d", p=128)
    out_r = out.rearrange("(to p) d -> p to d", p=128)   # [128, 8, 1024]

    with tile.TileContext(nc) as tc:
        with (
            tc.tile_pool(name="const", bufs=1) as constp,
            tc.tile_pool(name="xw", bufs=1) as xwp,
            tc.tile_pool(name="wts", bufs=3) as wp,
            tc.tile_pool(name="acts", bufs=1) as actp,
            tc.tile_pool(name="ptiles", bufs=16) as pp,
            tc.tile_pool(name="normp", bufs=4) as normp,
            tc.tile_pool(name="outs", bufs=4) as op,
            tc.tile_pool(name="psA", bufs=2, space="PSUM") as psA,
            tc.tile_pool(name="psS", bufs=2, space="PSUM") as psS,
            tc.tile_pool(name="psV", bufs=1, space="PSUM") as psV,
        ):
            # ---- inputs, ordered by first need ----
            bq_sb = constp.tile([128, 8], f32)
            bk_sb = constp.tile([128, 8], f32)
            vones_sb = constp.tile([128, 12], f32)

            x_sb = xwp.tile([128, 8, NKV], bf16)         # 24 KB/part
            wq_sb = wp.tile([128, 8, D], bf16, tag="w")
            wk_sb = wp.tile([128, 8, D], bf16, tag="w")
            wv_sb = wp.tile([128, 8, D], bf16, tag="w")
            # three parallel load streams ordered by first need: the
            # critical wq/wk + x stripe 1 split across both HWDGE queues,
            # later x stripes on the GPSIMD software DGE
            nc.scalar.dma_start(bq_sb[:], bqr[:])
            for ko in range(8):
                nc.sync.dma_start(wq_sb[:, ko], wq_r[:, ko])
                nc.gpsimd.dma_start(x_sb[:, ko, 512:1024],
                                    xT_r[:, ko, 512:1024])
            nc.scalar.dma_start(bk_sb[:], bkr[:])
            nc.scalar.dma_start(vones_sb[:], vones[:])
            for ko in range(8):
                nc.sync.dma_start(wk_sb[:, ko], wk_r[:, ko])
                nc.gpsimd.dma_start(x_sb[:, ko, 0:512], xT_r[:, ko, 0:512])
            bo_sb = constp.tile([128, D], bf16)
            nc.scalar.dma_start(bo_sb[:], borep[:])
            for ko in range(8):
                nc.sync.dma_start(wv_sb[:, ko], wv_r[:, ko])
                nc.gpsimd.dma_start(x_sb[:, ko, 1024:1536],
                                    xT_r[:, ko, 1024:1536])
            x_mm = x_sb[:]

            bv_sb = constp.tile([128, D], bf16)
            nc.scalar.dma_start(bv_sb[:], bvrep[:])
            wo_box = {}

            # ---- persistent activations ----
            q_sb = actp.tile([128, 8, BLK], bf16, tag="q")    # q^T [d, tok]
            k_sb = actp.tile([128, 8, NKV], bf16, tag="k")    # k^T [d, tok]
            v_sb = actp.tile([128, 12, H * (DH + 1)], bf16, tag="v")
            # ctx natural [q-part, j, head, dh], one chunk at a time
            ctxn_sb = actp.tile([128, 4, H, DH], bf16, tag="ctxn")
            ctxT_sb = actp.tile([128, 8, BLK], bf16, tag="ctxT")  # ctx^T

            v_ones = v_sb[:].rearrange("p t (h e) -> p t h e", e=DH + 1)

            # ---- thunk-granular work queue dripped into attention ----
            work = deque()

            def drip(n):
                for _ in range(n):
                    if work:
                        work.popleft()()

            def g_proj_qk(w_mm, dst, bias, xn, dn, m):
                """8 matmul thunks for one m-tile of a q^T/k^T stripe; the
                last thunk also adds the bias (per-partition scalar)."""
                box = {}

                def mk(ko):
                    def f():
                        if ko == 0:
                            box["ps"] = psA.tile([128, 512], f32,
                                                 name="ps", tag="ps")
                        nc.tensor.matmul(
                            box["ps"][:],
                            w_mm[:, ko, m * 128:(m + 1) * 128],
                            x_mm[:, ko, xn * 512:(xn + 1) * 512],
                            start=(ko == 0), stop=(ko == 7),
                        )
                        if ko == 7:
                            nc.vector.tensor_scalar_add(
                                dst[:, m, dn * 512:dn * 512 + 512],
                                box["ps"][:], bias[:, m:m + 1],
                            )
                    return f
                return [mk(ko) for ko in range(8)]

            def g_proj_v(t, n):
                """8 matmul thunks for one [128-token x 8-head] v tile; the
                last adds bias and zeroes halo rows."""
                box = {}

                def mk(ko):
                    def f():
                        if ko == 0:
                            box["ps"] = psA.tile([128, 512], f32,
                                                 name="ps", tag="ps")
                        nc.tensor.matmul(
                            box["ps"][:],
                            x_mm[:, ko, t * 128:(t + 1) * 128],
                            wv_sb[:, ko, n * 512:(n + 1) * 512],
                            start=(ko == 0), stop=(ko == 7),
                        )
                        if ko == 7:
                            dst = v_ones[:, t, n * 8:(n + 1) * 8, :DH]
                            nc.vector.tensor_add(
                                dst,
                                box["ps"][:].rearrange("p (h e) -> p h e",
                                                       e=DH),
                                bv_sb[:, n * 512:(n + 1) * 512]
                                .rearrange("p (h e) -> p h e", e=DH),
                            )
                            if t < 4:
                                nc.vector.tensor_scalar_mul(
                                    dst, dst, vones_sb[:, t:t + 1])
                    return f
                return [mk(ko) for ko in range(8)]

            def g_out_proj(to, n):
                """8 matmul thunks for one [128-token x 512] out tile; the
                last adds bias and stores."""
                box = {}

                def mk(ko):
                    def f():
                        if ko == 0:
                            box["ps"] = psA.tile([128, 512], f32,
                                                 name="ps", tag="ps")
                        nc.tensor.matmul(
                            box["ps"][:],
                            ctxT_sb[:, ko, to * 128:(to + 1) * 128],
                            wo_box["t"][:, ko, n * 512:(n + 1) * 512],
                            start=(ko == 0), stop=(ko == 7),
                        )
                        if ko == 7:
                            o_t = op.tile([128, 512], f32, tag="o")
                            nc.vector.tensor_add(
                                o_t[:], box["ps"][:],
                                bo_sb[:, n * 512:(n + 1) * 512])
                            # SWDGE store: keeps the exp-busy Act sequencer
                            # and the transpose-busy SP sequencer free
                            nc.gpsimd.dma_start(
                                out_r[:, to, n * 512:(n + 1) * 512], o_t[:])
                    return f
                return [mk(ko) for ko in range(8)]

            def attn_scores(c, u, drips, avq):
                """Chunk c, head pair (2u, 2u+1): per k-tile, the two heads'
                transposed scores share one 2-bank PSUM tile and one exp.
                Each slot also runs one deferred A*V group of the PREVIOUS
                pair (from avq) plus drips[i] fill thunks, keeping the PE
                dense through the exp-paced stretch. Returns the pair's 8
                exp'd p tiles."""
                qsl = slice(c * 512, (c + 1) * 512)
                pts = []
                for i in range(KT):
                    ksl = slice(c * 512 + i * 128, c * 512 + (i + 1) * 128)
                    sps = psS.tile([128, 1024], f32, name="sps")
                    nc.tensor.matmul(sps[:, 0:512],
                                     k_sb[0:64, u, ksl], q_sb[0:64, u, qsl],
                                     start=True, stop=True)
                    nc.tensor.matmul(sps[:, 512:1024],
                                     k_sb[64:128, u, ksl], q_sb[64:128, u, qsl],
                                     start=True, stop=True)
                    p_t = pp.tile([128, 1024], bf16, tag="p")
                    nc.scalar.activation(p_t[:], sps[:], AF.Exp, scale=0.125)
                    pts.append(p_t)
                    if avq:
                        avq.popleft()()
                    drip(drips[i])
                return pts

            def make_av_groups(c, u, pts):
                """A*V for pair (c, u), natural orientation, as 8 thunks (one
                per (q-subtile j, head)). Each thunk's 8 accumulating matmuls
                are contiguous, as hardware PSUM accumulation requires groups
                in one bank not to interleave; groups in different banks (e
                vs o, scores, fills) may. Consumed one per slot during the
                NEXT pair's attn_scores."""
                hs_e = (2 * u) * (DH + 1)
                hs_o = (2 * u + 1) * (DH + 1)
                box = {}

                def g(j, par):
                    def f():
                        if j == 0 and par == 0:
                            box["e"] = psV.tile([128, 4, DH + 1], f32,
                                                name="av_e", tag="av_e")
                            box["o"] = psV.tile([128, 4, DH + 1], f32,
                                                name="av_o", tag="av_o")
                        av = box["e"] if par == 0 else box["o"]
                        hs = hs_e if par == 0 else hs_o
                        off = 512 * par
                        for i in range(KT):
                            nc.tensor.matmul(
                                av[:, j],
                                pts[i][:, off + j * 128:off + (j + 1) * 128],
                                v_sb[:, 4 * c + i, hs:hs + DH + 1],
                                start=(i == 0), stop=(i == KT - 1),
                            )
                    return f
                return deque(g(j, par) for j in range(4) for par in (0, 1)), box

            def drain_pair(c, u, box, fine=False):
                """Normalize by the denominator column while copying psum ->
                sbuf (natural ctx, bf16), then transpose this pair's 128
                d-columns per q-tile to ctx^T via XBAR DMA
                (out[p, t] = in[t, p]). fine=True drains q-tile 0 first and
                issues its transpose immediately (tail-latency path)."""
                jsets = [(slice(0, 1), [0]), (slice(1, 4), [1, 2, 3])] \
                    if fine else [(slice(0, 4), [0, 1, 2, 3])]
                for jsl, js in jsets:
                    nj = len(js)
                    for par, key in ((0, "e"), (1, "o")):
                        h = 2 * u + par
                        av = box[key]
                        rec = normp.tile([128, nj, 1], f32, tag="rec")
                        nc.vector.reciprocal(rec[:], av[:, jsl, DH:DH + 1])
                        nc.vector.tensor_mul(
                            ctxn_sb[:, jsl, h, :],
                            av[:, jsl, 0:DH],
                            rec[:].to_broadcast([128, nj, DH]),
                        )
                    for j in js:
                        nc.sync.dma_start_transpose(
                            ctxT_sb[:, u,
                                    c * 512 + j * 128:c * 512 + (j + 1) * 128],
                            ctxn_sb[:, j, 2 * u:2 * u + 2, :]
                            .rearrange("p h e -> p (h e)"),
                        )

            # ---- prelude: chunk-0 attention deps + q stripe 1 (so wq is
            # free for wo's pool slot before the output projection) ----
            for m in range(8):                    # q stripe 0 (chunk-0 qs)
                for f in g_proj_qk(wq_sb[:], q_sb, bq_sb, 1, 0, m):
                    f()
            for m in range(8):                    # k stripe 1 (reuses x s1)
                for f in g_proj_qk(wk_sb[:], k_sb, bk_sb, 1, 1, m):
                    f()
            # validity column per head: 1 for valid keys, 0 for halo keys
            nc.vector.tensor_copy(
                v_ones[:, :, :, DH],
                vones_sb[:, :, None].to_broadcast([128, 12, H]),
            )
            for m in range(8):                    # k stripe 0 (halo)
                for f in g_proj_qk(wk_sb[:], k_sb, bk_sb, 0, 0, m):
                    f()
            for t in range(8):                    # v heads 0..7, kv tiles 0..7
                for f in g_proj_v(t, 0):
                    f()

            # ---- chunk-0 fill queue: things chunk 1 needs ----
            for t in range(8):
                work.extend(g_proj_v(t, 1))       # v heads 8..15, tiles 0..7
            for t in range(8, 12):
                work.extend(g_proj_v(t, 0))       # v heads 0..7, tiles 8..11
            work.extend(g_proj_qk(wk_sb[:], k_sb, bk_sb, 2, 2, 0))  # k2 m0
            work.extend(g_proj_qk(wk_sb[:], k_sb, bk_sb, 2, 2, 1))  # k2 m1
            work.extend(g_proj_qk(wq_sb[:], q_sb, bq_sb, 2, 1, 0))  # q1 m0
            # ---- software-pipelined attention: pair u's A*V groups run
            # inside pair u+1's score/exp slots ----
            pend = None                           # (avq, box, c, u)
            for c in range(CHUNKS):
                if c == 1:
                    # chunk-1 fill queue: q stripe 1 first (pair u's scores
                    # need q1 m_u; m0 came from the chunk-0 queue), then the
                    # rest of k2 / v n=1 tiles 8-11 / chunk-0 out projection
                    work.extend(g_proj_qk(wq_sb[:], q_sb, bq_sb, 2, 1, 1))
                    for m in range(2, 5):
                        work.extend(g_proj_qk(wq_sb[:], q_sb, bq_sb, 2, 1, m))
                        work.extend(g_proj_qk(wk_sb[:], k_sb, bk_sb, 2, 2, m))
                    for m in range(5, 8):
                        work.extend(g_proj_qk(wq_sb[:], q_sb, bq_sb, 2, 1, m))
                    for m in range(5, 8):
                        work.extend(g_proj_qk(wk_sb[:], k_sb, bk_sb, 2, 2, m))
                    for t in range(8, 12):
                        work.extend(g_proj_v(t, 1))
                    # wo reuses wq's pool slot (free once q1 m7 runs); the
                    # loads go via the software DGE so they cannot head-of-
                    # line-block the transpose stream on sync
                    wo_sb = wp.tile([128, 8, D], bf16, tag="w")
                    for ko in range(8):
                        nc.gpsimd.dma_start(wo_sb[:, ko], wo_r[:, ko])
                    wo_box["t"] = wo_sb
                    for to in range(4):
                        for n in range(2):
                            work.extend(g_out_proj(to, n))
                for u in range(8):
                    avq = pend[0] if pend else deque()
                    if c == 0:
                        drips = [3] * 8 if u == 0 else [2] * 8
                    else:
                        drips = [3] * 8 if u < 6 else [2] * 8
                    pts = attn_scores(c, u, drips, avq)
                    if pend:
                        drain_pair(pend[2], pend[3], pend[1])
                    avq, box = make_av_groups(c, u, pts)
                    pend = (avq, box, c, u)

            # ---- tail: last pair's A*V + drain, remaining fills bridge the
            # drain/transpose latency, then the chunk-1 output projection ----
            for f in pend[0]:
                f()
                drip(4)
            drain_pair(pend[2], pend[3], pend[1], fine=True)
            while work:
                work.popleft()()
            for to in range(4, 8):
                for n in range(2):
                    for f in g_out_proj(to, n):
                        f()

    nc.compile()
    return nc


def _host_prep(x, Wq, bq, Wk, bk, Wv, bv, Wo, bo):
    import ml_dtypes

    bf = ml_dtypes.bfloat16
    x = np.ascontiguousarray(np.asarray(x, dtype=np.float32))
    mats = {
        "wqT": np.ascontiguousarray(np.asarray(Wq, np.float32).T.astype(bf)),
        "wkT": np.ascontiguousarray(np.asarray(Wk, np.float32).T.astype(bf)),
        "wvT": np.ascontiguousarray(np.asarray(Wv, np.float32).T.astype(bf)),
        "woT": np.ascontiguousarray(np.asarray(Wo, np.float32).T.astype(bf)),
        "bqr": np.ascontiguousarray(
            np.asarray(bq, np.float32).reshape(8, 128).T),
        "bkr": np.ascontiguousarray(
            np.asarray(bk, np.float32).reshape(8, 128).T),
        "bvrep": np.ascontiguousarray(
            np.tile(np.asarray(bv, np.float32)[None, :], (128, 1)).astype(bf)),
        "borep": np.ascontiguousarray(
            np.tile(np.asarray(bo, np.float32)[None, :], (128, 1)).astype(bf)),
    }

    in_maps = []
    for core in range(NCORES):
        b, j = core // 4, core % 4
        start = j * BLK
        xkv = np.zeros((NKV, D), np.float32)
        lo = start - W
        if lo < 0:
            xkv[W:] = x[b, start:start + BLK]
        else:
            xkv[:] = x[b, lo:start + BLK]
        vo = np.ones((128, 12), np.float32)
        if j == 0:
            vo[:, 0:4] = 0.0         # halo keys (tokens 0..511) are invalid
        im = dict(mats)
        im["xT"] = np.ascontiguousarray(xkv.T.astype(bf))
        im["vones"] = vo
        in_maps.append(im)
    return in_maps


def kernel(x, Wq, bq, Wk, bk, Wv, bv, Wo, bo):
    from concourse.bass_utils import run_bass_kernel_spmd

    if "nc" not in _CACHE:
        _CACHE["nc"] = _build()
    nc = _CACHE["nc"]

    in_maps = _host_prep(x, Wq, bq, Wk, bk, Wv, bv, Wo, bo)
    res = run_bass_kernel_spmd(nc, in_maps, list(range(NCORES)))

    out = np.empty((B, L, D), np.float32)
    for core in range(NCORES):
        b, j = core // 4, core % 4
        out[b, j * BLK:(j + 1) * BLK] = res.results[core]["out"]
    return out


# revision 27
# speedup vs baseline: 1.0441x; 1.0151x over previous
"""Longformer (chunked sliding-window) self-attention on 8 TRN2 NeuronCores.

Sharding: sequence-parallel. B=2, L=4096 -> 8 blocks of 1024 query tokens
(4 blocks per batch element), one block per core. Each core also receives a
512-token K/V halo (the previous chunk), so no cross-core communication is
needed. The first block of each batch gets a zero halo; halo keys are made
invalid not by an additive mask but by a per-key validity column in V, which
drops them from both softmax numerator and denominator exactly like the
reference's -1e9 masking.

On-chip layout choices (per core):
  - x is passed pre-transposed (xT [D, NKV], bf16); weights pre-transposed
    (W.T, [din, dout], bf16).
  - q, k are produced transposed ([d, tok], bf16); v natural ([tok, d], bf16)
    with a validity column appended per head (1 valid / 0 halo).
  - scores are computed transposed (k_tok on psum partitions); two heads of a
    pair share one [128, 1024] 2-bank PSUM tile so one ScalarE exp covers both.
  - A*V runs in NATURAL orientation: lhsT = p^T tile [k,128q] (stationary),
    rhs = v tile [k, 65] (64 dims + validity column) -> psum [128q, 65]
    accumulated over the 8 k-tiles. This halves the PE cost vs the transposed
    formulation (moving dim 65 instead of 512 per head) and lands the softmax
    denominator in column 64 of the same psum tile.
  - softmax division fuses into the psum drain: per-partition reciprocal of
    column 64, then one broadcast multiply per head writes normalized ctx
    (natural [tok, d], bf16) to SBUF. No selection matmuls, no denominator
    gather DMAs.
  - ctx is transposed for the output projection by XBAR DMA-transpose
    (SBUF->SBUF, one [128q x 128d] instruction per (q-tile, head pair),
    emitted right after that pair's drain), costing no PE or DVE cycles.
  - Scheduling: the attention exp stream is the per-chunk pacing item, so all
    non-prelude projection work (q stripe 1, k stripe 2, v tiles 8-11, the
    n=1 half of v, and the chunk-0 output projection) is chopped into
    single-matmul thunks and dripped into the attention loop between k-tile
    iterations, keeping the PE dense (which also keeps it at full p-state).
"""

from collections import deque

import numpy as np

B, L, D = 2, 4096, 1024
H, DH, W = 16, 64, 512
NCORES = 8
BLK = L // 4          # 1024 query tokens per core
NKV = BLK + W         # 1536 kv tokens (halo + own)
CHUNKS = BLK // W     # 2 chunks per core
KT = (2 * W) // 128   # 8 k-token tiles of 128 per chunk window

_CACHE = {}


def _build():
    import concourse.bacc as bacc
    import concourse.mybir as mybir
    import concourse.tile as tile

    f32 = mybir.dt.float32
    bf16 = mybir.dt.bfloat16
    AF = mybir.ActivationFunctionType

    nc = bacc.Bacc("TRN2", target_bir_lowering=False, debug=False,
                   num_devices=NCORES)

    xT = nc.dram_tensor("xT", [D, NKV], bf16, kind="ExternalInput").ap()
    wqT = nc.dram_tensor("wqT", [D, D], bf16, kind="ExternalInput").ap()
    wkT = nc.dram_tensor("wkT", [D, D], bf16, kind="ExternalInput").ap()
    wvT = nc.dram_tensor("wvT", [D, D], bf16, kind="ExternalInput").ap()
    woT = nc.dram_tensor("woT", [D, D], bf16, kind="ExternalInput").ap()
    bqr = nc.dram_tensor("bqr", [128, 8], f32, kind="ExternalInput").ap()
    bkr = nc.dram_tensor("bkr", [128, 8], f32, kind="ExternalInput").ap()
    bvrep = nc.dram_tensor("bvrep", [128, D], bf16, kind="ExternalInput").ap()
    borep = nc.dram_tensor("borep", [128, D], bf16, kind="ExternalInput").ap()
    vones = nc.dram_tensor("vones", [128, 12], f32, kind="ExternalInput").ap()
    out = nc.dram_tensor("out", [BLK, D], f32, kind="ExternalOutput").ap()

    xT_r = xT.rearrange("(ko p) t -> p ko t", p=128)     # [128, 8, 1536]
    wq_r = wqT.rearrange("(ko p) d -> p ko d", p=128)    # [128, 8, 1024]
    wk_r = wkT.rearrange("(ko p) d -> p ko d", p=128)
    wv_r = wvT.rearrange("(ko p) d -> p ko d", p=128)
    wo_r = woT.rearrange("(ko p) d -> p ko d", p=128)
    out_r = out.rearrange("(to p) d -> p to d", p=128)   # [128, 8, 1024]

    with tile.TileContext(nc) as tc:
        with (
            tc.tile_pool(name="const", bufs=1) as constp,
            tc.tile_pool(name="xw", bufs=1) as xwp,
            tc.tile_pool(name="wts", bufs=3) as wp,
            tc.tile_pool(name="acts", bufs=1) as actp,
            tc.tile_pool(name="ptiles", bufs=16) as pp,
            tc.tile_pool(name="normp", bufs=4) as normp,
            tc.tile_pool(name="outs", bufs=4) as op,
            tc.tile_pool(name="psA", bufs=2, space="PSUM") as psA,
            tc.tile_pool(name="psS", bufs=2, space="PSUM") as psS,
            tc.tile_pool(name="psV", bufs=1, space="PSUM") as psV,
        ):
            # ---- inputs, ordered by first need ----
            bq_sb = constp.tile([128, 8], f32)
            bk_sb = constp.tile([128, 8], f32)
            vones_sb = constp.tile([128, 12], f32)

            x_sb = xwp.tile([128, 8, NKV], bf16)         # 24 KB/part
            wq_sb = wp.tile([128, 8, D], bf16, tag="w")
            wk_sb = wp.tile([128, 8, D], bf16, tag="w")
            wv_sb = wp.tile([128, 8, D], bf16, tag="w")
            # three parallel load streams ordered by first need: the
            # critical wq/wk + x stripe 1 split across both HWDGE queues,
            # later x stripes on the GPSIMD software DGE
            nc.scalar.dma_start(bq_sb[:], bqr[:])
            for ko in range(8):
                nc.sync.dma_start(wq_sb[:, ko], wq_r[:, ko])
                nc.gpsimd.dma_start(x_sb[:, ko, 512:1024],
                                    xT_r[:, ko, 512:1024])
            nc.scalar.dma_start(bk_sb[:], bkr[:])
            nc.scalar.dma_start(vones_sb[:], vones[:])
            for ko in range(8):
                nc.sync.dma_start(wk_sb[:, ko], wk_r[:, ko])
                nc.gpsimd.dma_start(x_sb[:, ko, 0:512], xT_r[:, ko, 0:512])
            bo_sb = constp.tile([128, D], bf16)
            nc.scalar.dma_start(bo_sb[:], borep[:])
            for ko in range(8):
                nc.sync.dma_start(wv_sb[:, ko], wv_r[:, ko])
                nc.gpsimd.dma_start(x_sb[:, ko, 1024:1536],
                                    xT_r[:, ko, 1024:1536])
            x_mm = x_sb[:]

            bv_sb = constp.tile([128, D], bf16)
            nc.scalar.dma_start(bv_sb[:], bvrep[:])
            wo_box = {}

            # ---- persistent activations ----
            q_sb = actp.tile([128, 8, BLK], bf16, tag="q")    # q^T [d, tok]
            k_sb = actp.tile([128, 8, NKV], bf16, tag="k")    # k^T [d, tok]
            v_sb = actp.tile([128, 12, H * (DH + 1)], bf16, tag="v")
            # ctx natural [q-part, j, head, dh], one chunk at a time
            ctxn_sb = actp.tile([128, 4, H, DH], bf16, tag="ctxn")
            ctxT_sb = actp.tile([128, 8, BLK], bf16, tag="ctxT")  # ctx^T

            v_ones = v_sb[:].rearrange("p t (h e) -> p t h e", e=DH + 1)

            # ---- thunk-granular work queue dripped into attention ----
            work = deque()

            def drip(n):
                for _ in range(n):
                    if work:
                        work.popleft()()

            def g_proj_qk(w_mm, dst, bias, xn, dn, m):
                """8 matmul thunks for one m-tile of a q^T/k^T stripe; the
                last thunk also adds the bias (per-partition scalar)."""
                box = {}

                def mk(ko):
                    def f():
                        if ko == 0:
                            box["ps"] = psA.tile([128, 512], f32,
                                                 name="ps", tag="ps")
                        nc.tensor.matmul(
                            box["ps"][:],
                            w_mm[:, ko, m * 128:(m + 1) * 128],
                            x_mm[:, ko, xn * 512:(xn + 1) * 512],
                            start=(ko == 0), stop=(ko == 7),
                        )
                        if ko == 7:
                            nc.vector.tensor_scalar_add(
                                dst[:, m, dn * 512:dn * 512 + 512],
                                box["ps"][:], bias[:, m:m + 1],
                            )
                    return f
                return [mk(ko) for ko in range(8)]

            def g_proj_v(t, n):
                """8 matmul thunks for one [128-token x 8-head] v tile; the
                last adds bias and zeroes halo rows."""
                box = {}

                def mk(ko):
                    def f():
                        if ko == 0:
                            box["ps"] = psA.tile([128, 512], f32,
                                                 name="ps", tag="ps")
                        nc.tensor.matmul(
                            box["ps"][:],
                            x_mm[:, ko, t * 128:(t + 1) * 128],
                            wv_sb[:, ko, n * 512:(n + 1) * 512],
                            start=(ko == 0), stop=(ko == 7),
                        )
                        if ko == 7:
                            dst = v_ones[:, t, n * 8:(n + 1) * 8, :DH]
                            nc.vector.tensor_add(
                                dst,
                                box["ps"][:].rearrange("p (h e) -> p h e",
                                                       e=DH),
                                bv_sb[:, n * 512:(n + 1) * 512]
                                .rearrange("p (h e) -> p h e", e=DH),
                            )
                            if t < 4:
                                nc.vector.tensor_scalar_mul(
                                    dst, dst, vones_sb[:, t:t + 1])
                    return f
                return [mk(ko) for ko in range(8)]

            def g_out_proj(to, n, seng=None):
                """8 matmul thunks for one [128-token x 512] out tile; the
                last adds bias and stores via seng (default SWDGE)."""
                box = {}

                def mk(ko):
                    def f():
                        if ko == 0:
                            box["ps"] = psA.tile([128, 512], f32,
                                                 name="ps", tag="ps")
                        nc.tensor.matmul(
                            box["ps"][:],
                            ctxT_sb[:, ko, to * 128:(to + 1) * 128],
                            wo_box["t"][:, ko, n * 512:(n + 1) * 512],
                            start=(ko == 0), stop=(ko == 7),
                        )
                        if ko == 7:
                            o_t = op.tile([128, 512], f32, tag="o")
                            nc.vector.tensor_add(
                                o_t[:], box["ps"][:],
                                bo_sb[:, n * 512:(n + 1) * 512])
                            # mid-kernel stores go via SWDGE to keep the
                            # exp-busy Act sequencer and the transpose-busy
                            # SP sequencer free; tail stores rotate queues
                            (seng or nc.gpsimd).dma_start(
                                out_r[:, to, n * 512:(n + 1) * 512], o_t[:])
                    return f
                return [mk(ko) for ko in range(8)]

            def attn_scores(c, u, drips, avq):
                """Chunk c, head pair (2u, 2u+1): per k-tile, the two heads'
                transposed scores share one 2-bank PSUM tile and one exp.
                Each slot also runs one deferred A*V group of the PREVIOUS
                pair (from avq) plus drips[i] fill thunks, keeping the PE
                dense through the exp-paced stretch. Returns the pair's 8
                exp'd p tiles."""
                qsl = slice(c * 512, (c + 1) * 512)
                pts = []
                for i in range(KT):
                    ksl = slice(c * 512 + i * 128, c * 512 + (i + 1) * 128)
                    sps = psS.tile([128, 1024], f32, name="sps")
                    nc.tensor.matmul(sps[:, 0:512],
                                     k_sb[0:64, u, ksl], q_sb[0:64, u, qsl],
                                     start=True, stop=True)
                    nc.tensor.matmul(sps[:, 512:1024],
                                     k_sb[64:128, u, ksl], q_sb[64:128, u, qsl],
                                     start=True, stop=True)
                    p_t = pp.tile([128, 1024], bf16, tag="p")
                    nc.scalar.activation(p_t[:], sps[:], AF.Exp, scale=0.125)
                    pts.append(p_t)
                    if avq:
                        avq.popleft()()
                    drip(drips[i])
                return pts

            def make_av_groups(c, u, pts):
                """A*V for pair (c, u), natural orientation, as 8 thunks (one
                per (q-subtile j, head)). Each thunk's 8 accumulating matmuls
                are contiguous, as hardware PSUM accumulation requires groups
                in one bank not to interleave; groups in different banks (e
                vs o, scores, fills) may. Consumed one per slot during the
                NEXT pair's attn_scores."""
                hs_e = (2 * u) * (DH + 1)
                hs_o = (2 * u + 1) * (DH + 1)
                box = {}

                def g(j, par):
                    def f():
                        if j == 0 and par == 0:
                            box["e"] = psV.tile([128, 4, DH + 1], f32,
                                                name="av_e", tag="av_e")
                            box["o"] = psV.tile([128, 4, DH + 1], f32,
                                                name="av_o", tag="av_o")
                        av = box["e"] if par == 0 else box["o"]
                        hs = hs_e if par == 0 else hs_o
                        off = 512 * par
                        for i in range(KT):
                            nc.tensor.matmul(
                                av[:, j],
                                pts[i][:, off + j * 128:off + (j + 1) * 128],
                                v_sb[:, 4 * c + i, hs:hs + DH + 1],
                                start=(i == 0), stop=(i == KT - 1),
                            )
                    return f
                return deque(g(j, par) for j in range(4) for par in (0, 1)), box

            def drain_pair(c, u, box, fine=False):
                """Normalize by the denominator column while copying psum ->
                sbuf (natural ctx, bf16), then transpose this pair's 128
                d-columns per q-tile to ctx^T via XBAR DMA
                (out[p, t] = in[t, p]). fine=True drains q-tile 0 first and
                issues its transpose immediately (tail-latency path)."""
                jsets = [(slice(0, 1), [0]), (slice(1, 4), [1, 2, 3])] \
                    if fine else [(slice(0, 4), [0, 1, 2, 3])]
                for jsl, js in jsets:
                    nj = len(js)
                    for par, key in ((0, "e"), (1, "o")):
                        h = 2 * u + par
                        av = box[key]
                        rec = normp.tile([128, nj, 1], f32, tag="rec")
                        nc.vector.reciprocal(rec[:], av[:, jsl, DH:DH + 1])
                        nc.vector.tensor_mul(
                            ctxn_sb[:, jsl, h, :],
                            av[:, jsl, 0:DH],
                            rec[:].to_broadcast([128, nj, DH]),
                        )
                    for j in js:
                        nc.sync.dma_start_transpose(
                            ctxT_sb[:, u,
                                    c * 512 + j * 128:c * 512 + (j + 1) * 128],
                            ctxn_sb[:, j, 2 * u:2 * u + 2, :]
                            .rearrange("p h e -> p (h e)"),
                        )

            # ---- prelude: chunk-0 attention deps + q stripe 1 (so wq is
            # free for wo's pool slot before the output projection) ----
            for m in range(8):                    # q stripe 0 (chunk-0 qs)
                for f in g_proj_qk(wq_sb[:], q_sb, bq_sb, 1, 0, m):
                    f()
            for m in range(8):                    # k stripe 1 (reuses x s1)
                for f in g_proj_qk(wk_sb[:], k_sb, bk_sb, 1, 1, m):
                    f()
            # validity column per head: 1 for valid keys, 0 for halo keys
            nc.vector.tensor_copy(
                v_ones[:, :, :, DH],
                vones_sb[:, :, None].to_broadcast([128, 12, H]),
            )
            for m in range(8):                    # k stripe 0 (halo)
                for f in g_proj_qk(wk_sb[:], k_sb, bk_sb, 0, 0, m):
                    f()
            for t in range(8):                    # v heads 0..7, kv tiles 0..7
                for f in g_proj_v(t, 0):
                    f()

            # ---- chunk-0 fill queue: things chunk 1 needs ----
            for t in range(8):
                work.extend(g_proj_v(t, 1))       # v heads 8..15, tiles 0..7
            for t in range(8, 12):
                work.extend(g_proj_v(t, 0))       # v heads 0..7, tiles 8..11
            work.extend(g_proj_qk(wk_sb[:], k_sb, bk_sb, 2, 2, 0))  # k2 m0
            work.extend(g_proj_qk(wk_sb[:], k_sb, bk_sb, 2, 2, 1))  # k2 m1
            work.extend(g_proj_qk(wq_sb[:], q_sb, bq_sb, 2, 1, 0))  # q1 m0
            # ---- software-pipelined attention: pair u's A*V groups run
            # inside pair u+1's score/exp slots ----
            pend = None                           # (avq, box, c, u)
            for c in range(CHUNKS):
                if c == 1:
                    # chunk-1 fill queue: q stripe 1 first (pair u's scores
                    # need q1 m_u; m0 came from the chunk-0 queue), then the
                    # rest of k2 / v n=1 tiles 8-11 / chunk-0 out projection
                    work.extend(g_proj_qk(wq_sb[:], q_sb, bq_sb, 2, 1, 1))
                    for m in range(2, 5):
                        work.extend(g_proj_qk(wq_sb[:], q_sb, bq_sb, 2, 1, m))
                        work.extend(g_proj_qk(wk_sb[:], k_sb, bk_sb, 2, 2, m))
                    for m in range(5, 8):
                        work.extend(g_proj_qk(wq_sb[:], q_sb, bq_sb, 2, 1, m))
                    for t in range(8, 12):
                        work.extend(g_proj_v(t, 1))
                    for m in range(5, 8):
                        work.extend(g_proj_qk(wk_sb[:], k_sb, bk_sb, 2, 2, m))
                    # wo reuses wq's pool slot (free once q1 m7 runs); the
                    # loads go via the software DGE so they cannot head-of-
                    # line-block the transpose stream on sync
                    wo_sb = wp.tile([128, 8, D], bf16, tag="w")
                    for ko in range(8):
                        nc.gpsimd.dma_start(wo_sb[:, ko], wo_r[:, ko])
                    wo_box["t"] = wo_sb
                    for to in range(4):
                        for n in range(2):
                            work.extend(g_out_proj(to, n))
                for u in range(8):
                    avq = pend[0] if pend else deque()
                    if c == 0:
                        drips = [3] * 8 if u == 0 else [2] * 8
                    else:
                        drips = [3] * 8 if u < 5 else [2] * 8
                    pts = attn_scores(c, u, drips, avq)
                    if pend:
                        drain_pair(pend[2], pend[3], pend[1])
                    avq, box = make_av_groups(c, u, pts)
                    pend = (avq, box, c, u)

            # ---- tail: last pair's A*V + drain, remaining fills bridge the
            # drain/transpose latency, then the chunk-1 output projection ----
            for f in pend[0]:
                f()
                drip(4)
            drain_pair(pend[2], pend[3], pend[1], fine=True)
            while work:
                work.popleft()()
            engs = [None, nc.sync, nc.scalar]
            for to in range(4, 8):
                for n in range(2):
                    for f in g_out_proj(to, n, engs[(2 * to + n) % 3]):
                        f()

    nc.compile()
    return nc


def _host_prep(x, Wq, bq, Wk, bk, Wv, bv, Wo, bo):
    import ml_dtypes

    bf = ml_dtypes.bfloat16
    x = np.ascontiguousarray(np.asarray(x, dtype=np.float32))
    mats = {
        "wqT": np.ascontiguousarray(np.asarray(Wq, np.float32).T.astype(bf)),
        "wkT": np.ascontiguousarray(np.asarray(Wk, np.float32).T.astype(bf)),
        "wvT": np.ascontiguousarray(np.asarray(Wv, np.float32).T.astype(bf)),
        "woT": np.ascontiguousarray(np.asarray(Wo, np.float32).T.astype(bf)),
        "bqr": np.ascontiguousarray(
            np.asarray(bq, np.float32).reshape(8, 128).T),
        "bkr": np.ascontiguousarray(
            np.asarray(bk, np.float32).reshape(8, 128).T),
        "bvrep": np.ascontiguousarray(
            np.tile(np.asarray(bv, np.float32)[None, :], (128, 1)).astype(bf)),
        "borep": np.ascontiguousarray(
            np.tile(np.asarray(bo, np.float32)[None, :], (128, 1)).astype(bf)),
    }

    in_maps = []
    for core in range(NCORES):
        b, j = core // 4, core % 4
        start = j * BLK
        xkv = np.zeros((NKV, D), np.float32)
        lo = start - W
        if lo < 0:
            xkv[W:] = x[b, start:start + BLK]
        else:
            xkv[:] = x[b, lo:start + BLK]
        vo = np.ones((128, 12), np.float32)
        if j == 0:
            vo[:, 0:4] = 0.0         # halo keys (tokens 0..511) are invalid
        im = dict(mats)
        im["xT"] = np.ascontiguousarray(xkv.T.astype(bf))
        im["vones"] = vo
        in_maps.append(im)
    return in_maps


def kernel(x, Wq, bq, Wk, bk, Wv, bv, Wo, bo):
    from concourse.bass_utils import run_bass_kernel_spmd

    if "nc" not in _CACHE:
        _CACHE["nc"] = _build()
    nc = _CACHE["nc"]

    in_maps = _host_prep(x, Wq, bq, Wk, bk, Wv, bv, Wo, bo)
    res = run_bass_kernel_spmd(nc, in_maps, list(range(NCORES)))

    out = np.empty((B, L, D), np.float32)
    for core in range(NCORES):
        b, j = core // 4, core % 4
        out[b, j * BLK:(j + 1) * BLK] = res.results[core]["out"]
    return out


# revision 28
# speedup vs baseline: 1.0519x; 1.0075x over previous
"""Longformer (chunked sliding-window) self-attention on 8 TRN2 NeuronCores.

Sharding: sequence-parallel. B=2, L=4096 -> 8 blocks of 1024 query tokens
(4 blocks per batch element), one block per core. Each core also receives a
512-token K/V halo (the previous chunk), so no cross-core communication is
needed. The first block of each batch gets a zero halo; halo keys are made
invalid not by an additive mask but by a per-key validity column in V, which
drops them from both softmax numerator and denominator exactly like the
reference's -1e9 masking.

On-chip layout choices (per core):
  - x is passed pre-transposed (xT [D, NKV], bf16); weights pre-transposed
    (W.T, [din, dout], bf16).
  - q, k are produced transposed ([d, tok], bf16); v natural ([tok, d], bf16)
    with a validity column appended per head (1 valid / 0 halo).
  - scores are computed transposed (k_tok on psum partitions); two heads of a
    pair share one [128, 1024] 2-bank PSUM tile so one ScalarE exp covers both.
  - A*V runs in NATURAL orientation: lhsT = p^T tile [k,128q] (stationary),
    rhs = v tile [k, 65] (64 dims + validity column) -> psum [128q, 65]
    accumulated over the 8 k-tiles. This halves the PE cost vs the transposed
    formulation (moving dim 65 instead of 512 per head) and lands the softmax
    denominator in column 64 of the same psum tile.
  - softmax division fuses into the psum drain: per-partition reciprocal of
    column 64, then one broadcast multiply per head writes normalized ctx
    (natural [tok, d], bf16) to SBUF. No selection matmuls, no denominator
    gather DMAs.
  - ctx is transposed for the output projection by XBAR DMA-transpose
    (SBUF->SBUF, one [128q x 128d] instruction per (q-tile, head pair),
    emitted right after that pair's drain), costing no PE or DVE cycles.
  - Scheduling: the attention exp stream is the per-chunk pacing item, so all
    non-prelude projection work (q stripe 1, k stripe 2, v tiles 8-11, the
    n=1 half of v, and the chunk-0 output projection) is chopped into
    single-matmul thunks and dripped into the attention loop between k-tile
    iterations, keeping the PE dense (which also keeps it at full p-state).
"""

from collections import deque

import numpy as np

B, L, D = 2, 4096, 1024
H, DH, W = 16, 64, 512
NCORES = 8
BLK = L // 4          # 1024 query tokens per core
NKV = BLK + W         # 1536 kv tokens (halo + own)
CHUNKS = BLK // W     # 2 chunks per core
KT = (2 * W) // 128   # 8 k-token tiles of 128 per chunk window

_CACHE = {}


def _build():
    import concourse.bacc as bacc
    import concourse.mybir as mybir
    import concourse.tile as tile

    f32 = mybir.dt.float32
    bf16 = mybir.dt.bfloat16
    AF = mybir.ActivationFunctionType

    nc = bacc.Bacc("TRN2", target_bir_lowering=False, debug=False,
                   num_devices=NCORES)

    xT = nc.dram_tensor("xT", [D, NKV], bf16, kind="ExternalInput").ap()
    wqT = nc.dram_tensor("wqT", [D, D], bf16, kind="ExternalInput").ap()
    wkT = nc.dram_tensor("wkT", [D, D], bf16, kind="ExternalInput").ap()
    wvT = nc.dram_tensor("wvT", [D, D], bf16, kind="ExternalInput").ap()
    woT = nc.dram_tensor("woT", [D, D], bf16, kind="ExternalInput").ap()
    bqr = nc.dram_tensor("bqr", [128, 8], f32, kind="ExternalInput").ap()
    bkr = nc.dram_tensor("bkr", [128, 8], f32, kind="ExternalInput").ap()
    bvrep = nc.dram_tensor("bvrep", [128, D], bf16, kind="ExternalInput").ap()
    borep = nc.dram_tensor("borep", [128, D], bf16, kind="ExternalInput").ap()
    vones = nc.dram_tensor("vones", [128, 12], f32, kind="ExternalInput").ap()
    out = nc.dram_tensor("out", [BLK, D], f32, kind="ExternalOutput").ap()

    xT_r = xT.rearrange("(ko p) t -> p ko t", p=128)     # [128, 8, 1536]
    wq_r = wqT.rearrange("(ko p) d -> p ko d", p=128)    # [128, 8, 1024]
    wk_r = wkT.rearrange("(ko p) d -> p ko d", p=128)
    wv_r = wvT.rearrange("(ko p) d -> p ko d", p=128)
    wo_r = woT.rearrange("(ko p) d -> p ko d", p=128)
    out_r = out.rearrange("(to p) d -> p to d", p=128)   # [128, 8, 1024]

    with tile.TileContext(nc) as tc:
        with (
            tc.tile_pool(name="const", bufs=1) as constp,
            tc.tile_pool(name="xw", bufs=1) as xwp,
            tc.tile_pool(name="wts", bufs=3) as wp,
            tc.tile_pool(name="acts", bufs=1) as actp,
            tc.tile_pool(name="ptiles", bufs=16) as pp,
            tc.tile_pool(name="normp", bufs=4) as normp,
            tc.tile_pool(name="outs", bufs=4) as op,
            tc.tile_pool(name="psA", bufs=2, space="PSUM") as psA,
            tc.tile_pool(name="psS", bufs=2, space="PSUM") as psS,
            tc.tile_pool(name="psV", bufs=1, space="PSUM") as psV,
        ):
            # ---- inputs, ordered by first need ----
            bq_sb = constp.tile([128, 8], f32)
            bk_sb = constp.tile([128, 8], f32)
            vones_sb = constp.tile([128, 12], f32)

            x_sb = xwp.tile([128, 8, NKV], bf16)         # 24 KB/part
            wq_sb = wp.tile([128, 8, D], bf16, tag="w")
            wk_sb = wp.tile([128, 8, D], bf16, tag="w")
            wv_sb = wp.tile([128, 8, D], bf16, tag="w")
            # three parallel load streams ordered by first need: the
            # critical wq/wk + x stripe 1 split across both HWDGE queues,
            # later x stripes on the GPSIMD software DGE
            nc.scalar.dma_start(bq_sb[:], bqr[:])
            for ko in range(8):
                nc.sync.dma_start(wq_sb[:, ko], wq_r[:, ko])
                nc.gpsimd.dma_start(x_sb[:, ko, 512:1024],
                                    xT_r[:, ko, 512:1024])
            nc.scalar.dma_start(bk_sb[:], bkr[:])
            nc.scalar.dma_start(vones_sb[:], vones[:])
            for ko in range(8):
                nc.sync.dma_start(wk_sb[:, ko], wk_r[:, ko])
                nc.gpsimd.dma_start(x_sb[:, ko, 0:512], xT_r[:, ko, 0:512])
            bo_sb = constp.tile([128, D], bf16)
            nc.scalar.dma_start(bo_sb[:], borep[:])
            for ko in range(8):
                nc.sync.dma_start(wv_sb[:, ko], wv_r[:, ko])
                nc.gpsimd.dma_start(x_sb[:, ko, 1024:1536],
                                    xT_r[:, ko, 1024:1536])
            x_mm = x_sb[:]

            bv_sb = constp.tile([128, D], bf16)
            nc.scalar.dma_start(bv_sb[:], bvrep[:])
            wo_box = {}

            # ---- persistent activations ----
            q_sb = actp.tile([128, 8, BLK], bf16, tag="q")    # q^T [d, tok]
            k_sb = actp.tile([128, 8, NKV], bf16, tag="k")    # k^T [d, tok]
            v_sb = actp.tile([128, 12, H * (DH + 1)], bf16, tag="v")
            # ctx natural [q-part, j, head, dh], one chunk at a time
            ctxn_sb = actp.tile([128, 4, H, DH], bf16, tag="ctxn")
            ctxT_sb = actp.tile([128, 8, BLK], bf16, tag="ctxT")  # ctx^T

            v_ones = v_sb[:].rearrange("p t (h e) -> p t h e", e=DH + 1)

            # ---- thunk-granular work queue dripped into attention ----
            work = deque()

            def drip(n):
                for _ in range(n):
                    if work:
                        work.popleft()()

            def g_proj_qk(w_mm, dst, bias, xn, dn, m):
                """8 matmul thunks for one m-tile of a q^T/k^T stripe; the
                last thunk also adds the bias (per-partition scalar)."""
                box = {}

                def mk(ko):
                    def f():
                        if ko == 0:
                            box["ps"] = psA.tile([128, 512], f32,
                                                 name="ps", tag="ps")
                        nc.tensor.matmul(
                            box["ps"][:],
                            w_mm[:, ko, m * 128:(m + 1) * 128],
                            x_mm[:, ko, xn * 512:(xn + 1) * 512],
                            start=(ko == 0), stop=(ko == 7),
                        )
                        if ko == 7:
                            nc.vector.tensor_scalar_add(
                                dst[:, m, dn * 512:dn * 512 + 512],
                                box["ps"][:], bias[:, m:m + 1],
                            )
                    return f
                return [mk(ko) for ko in range(8)]

            def g_proj_v(t, n):
                """8 matmul thunks for one [128-token x 8-head] v tile; the
                last adds bias and zeroes halo rows."""
                box = {}

                def mk(ko):
                    def f():
                        if ko == 0:
                            box["ps"] = psA.tile([128, 512], f32,
                                                 name="ps", tag="ps")
                        nc.tensor.matmul(
                            box["ps"][:],
                            x_mm[:, ko, t * 128:(t + 1) * 128],
                            wv_sb[:, ko, n * 512:(n + 1) * 512],
                            start=(ko == 0), stop=(ko == 7),
                        )
                        if ko == 7:
                            dst = v_ones[:, t, n * 8:(n + 1) * 8, :DH]
                            nc.vector.tensor_add(
                                dst,
                                box["ps"][:].rearrange("p (h e) -> p h e",
                                                       e=DH),
                                bv_sb[:, n * 512:(n + 1) * 512]
                                .rearrange("p (h e) -> p h e", e=DH),
                            )
                            if t < 4:
                                nc.vector.tensor_scalar_mul(
                                    dst, dst, vones_sb[:, t:t + 1])
                    return f
                return [mk(ko) for ko in range(8)]

            def g_out_proj(to, n, seng=None):
                """8 matmul thunks for one [128-token x 512] out tile; the
                last adds bias and stores via seng (default SWDGE)."""
                box = {}

                def mk(ko):
                    def f():
                        if ko == 0:
                            box["ps"] = psA.tile([128, 512], f32,
                                                 name="ps", tag="ps")
                        nc.tensor.matmul(
                            box["ps"][:],
                            ctxT_sb[:, ko, to * 128:(to + 1) * 128],
                            wo_box["t"][:, ko, n * 512:(n + 1) * 512],
                            start=(ko == 0), stop=(ko == 7),
                        )
                        if ko == 7:
                            o_t = op.tile([128, 512], f32, tag="o")
                            nc.vector.tensor_add(
                                o_t[:], box["ps"][:],
                                bo_sb[:, n * 512:(n + 1) * 512])
                            # mid-kernel stores go via SWDGE to keep the
                            # exp-busy Act sequencer and the transpose-busy
                            # SP sequencer free; tail stores rotate queues
                            (seng or nc.gpsimd).dma_start(
                                out_r[:, to, n * 512:(n + 1) * 512], o_t[:])
                    return f
                return [mk(ko) for ko in range(8)]

            def attn_scores(c, u, drips, avq):
                """Chunk c, head pair (2u, 2u+1): per k-tile, the two heads'
                transposed scores share one 2-bank PSUM tile and one exp.
                Each slot also runs one deferred A*V group of the PREVIOUS
                pair (from avq) plus drips[i] fill thunks, keeping the PE
                dense through the exp-paced stretch. Returns the pair's 8
                exp'd p tiles."""
                qsl = slice(c * 512, (c + 1) * 512)
                pts = []
                for i in range(KT):
                    ksl = slice(c * 512 + i * 128, c * 512 + (i + 1) * 128)
                    sps = psS.tile([128, 1024], f32, name="sps")
                    nc.tensor.matmul(sps[:, 0:512],
                                     k_sb[0:64, u, ksl], q_sb[0:64, u, qsl],
                                     start=True, stop=True)
                    nc.tensor.matmul(sps[:, 512:1024],
                                     k_sb[64:128, u, ksl], q_sb[64:128, u, qsl],
                                     start=True, stop=True)
                    p_t = pp.tile([128, 1024], bf16, tag="p")
                    nc.scalar.activation(p_t[:], sps[:], AF.Exp, scale=0.125)
                    pts.append(p_t)
                    if avq:
                        avq.popleft()()
                    drip(drips[i])
                return pts

            def make_av_groups(c, u, pts):
                """A*V for pair (c, u), natural orientation, as 8 thunks (one
                per (q-subtile j, head)). Each thunk's 8 accumulating matmuls
                are contiguous, as hardware PSUM accumulation requires groups
                in one bank not to interleave; groups in different banks (e
                vs o, scores, fills) may. Consumed one per slot during the
                NEXT pair's attn_scores."""
                hs_e = (2 * u) * (DH + 1)
                hs_o = (2 * u + 1) * (DH + 1)
                box = {}

                def g(j, par):
                    def f():
                        if j == 0 and par == 0:
                            box["e"] = psV.tile([128, 4, DH + 1], f32,
                                                name="av_e", tag="av_e")
                            box["o"] = psV.tile([128, 4, DH + 1], f32,
                                                name="av_o", tag="av_o")
                        av = box["e"] if par == 0 else box["o"]
                        hs = hs_e if par == 0 else hs_o
                        off = 512 * par
                        for i in range(KT):
                            nc.tensor.matmul(
                                av[:, j],
                                pts[i][:, off + j * 128:off + (j + 1) * 128],
                                v_sb[:, 4 * c + i, hs:hs + DH + 1],
                                start=(i == 0), stop=(i == KT - 1),
                            )
                    return f
                return deque(g(j, par) for j in range(4) for par in (0, 1)), box

            def drain_pair(c, u, box, fine=False):
                """Normalize by the denominator column while copying psum ->
                sbuf (natural ctx, bf16), then transpose this pair's 128
                d-columns per q-tile to ctx^T via XBAR DMA
                (out[p, t] = in[t, p]). fine=True drains q-tile 0 first and
                issues its transpose immediately (tail-latency path)."""
                jsets = [(slice(0, 1), [0]), (slice(1, 4), [1, 2, 3])] \
                    if fine else [(slice(0, 4), [0, 1, 2, 3])]
                for jsl, js in jsets:
                    nj = len(js)
                    for par, key in ((0, "e"), (1, "o")):
                        h = 2 * u + par
                        av = box[key]
                        rec = normp.tile([128, nj, 1], f32, tag="rec")
                        nc.vector.reciprocal(rec[:], av[:, jsl, DH:DH + 1])
                        nc.vector.tensor_mul(
                            ctxn_sb[:, jsl, h, :],
                            av[:, jsl, 0:DH],
                            rec[:].to_broadcast([128, nj, DH]),
                        )
                    for j in js:
                        nc.sync.dma_start_transpose(
                            ctxT_sb[:, u,
                                    c * 512 + j * 128:c * 512 + (j + 1) * 128],
                            ctxn_sb[:, j, 2 * u:2 * u + 2, :]
                            .rearrange("p h e -> p (h e)"),
                        )

            # ---- prelude: chunk-0 attention deps + q stripe 1 (so wq is
            # free for wo's pool slot before the output projection) ----
            for m in range(8):                    # q stripe 0 (chunk-0 qs)
                for f in g_proj_qk(wq_sb[:], q_sb, bq_sb, 1, 0, m):
                    f()
            for m in range(8):                    # k stripe 1 (reuses x s1)
                for f in g_proj_qk(wk_sb[:], k_sb, bk_sb, 1, 1, m):
                    f()
            # validity column per head: 1 for valid keys, 0 for halo keys
            nc.vector.tensor_copy(
                v_ones[:, :, :, DH],
                vones_sb[:, :, None].to_broadcast([128, 12, H]),
            )
            for m in range(8):                    # k stripe 0 (halo)
                for f in g_proj_qk(wk_sb[:], k_sb, bk_sb, 0, 0, m):
                    f()
            for t in range(8):                    # v heads 0..7, kv tiles 0..7
                for f in g_proj_v(t, 0):
                    f()

            # ---- chunk-0 fill queue: things chunk 1 needs ----
            for t in range(8):
                work.extend(g_proj_v(t, 1))       # v heads 8..15, tiles 0..7
            for t in range(8, 12):
                work.extend(g_proj_v(t, 0))       # v heads 0..7, tiles 8..11
            work.extend(g_proj_qk(wk_sb[:], k_sb, bk_sb, 2, 2, 0))  # k2 m0
            work.extend(g_proj_qk(wk_sb[:], k_sb, bk_sb, 2, 2, 1))  # k2 m1
            work.extend(g_proj_qk(wq_sb[:], q_sb, bq_sb, 2, 1, 0))  # q1 m0
            # ---- software-pipelined attention: pair u's A*V groups run
            # inside pair u+1's score/exp slots ----
            pend = None                           # (avq, box, c, u)
            for c in range(CHUNKS):
                if c == 1:
                    # chunk-1 fill queue: q stripe 1 first (pair u's scores
                    # need q1 m_u; m0 came from the chunk-0 queue), then the
                    # rest of k2 / v n=1 tiles 8-11 / chunk-0 out projection
                    work.extend(g_proj_qk(wq_sb[:], q_sb, bq_sb, 2, 1, 1))
                    for m in range(2, 5):
                        work.extend(g_proj_qk(wq_sb[:], q_sb, bq_sb, 2, 1, m))
                        work.extend(g_proj_qk(wk_sb[:], k_sb, bk_sb, 2, 2, m))
                    for m in range(5, 8):
                        work.extend(g_proj_qk(wq_sb[:], q_sb, bq_sb, 2, 1, m))
                    for t in range(8, 12):
                        work.extend(g_proj_v(t, 1))
                    for m in range(5, 8):
                        work.extend(g_proj_qk(wk_sb[:], k_sb, bk_sb, 2, 2, m))
                    # wo reuses wq's pool slot (free once q1 m7 runs); the
                    # loads go via the software DGE so they cannot head-of-
                    # line-block the transpose stream on sync
                    wo_sb = wp.tile([128, 8, D], bf16, tag="w")
                    for ko in range(8):
                        nc.gpsimd.dma_start(wo_sb[:, ko], wo_r[:, ko])
                    wo_box["t"] = wo_sb
                    for to in range(4):
                        for n in range(2):
                            work.extend(g_out_proj(to, n))
                for u in range(8):
                    avq = pend[0] if pend else deque()
                    if c == 0:
                        drips = [3] * 8 if u == 0 else [2] * 8
                    else:
                        drips = [3] * 8 if u < 5 else [2] * 8
                    pts = attn_scores(c, u, drips, avq)
                    if pend:
                        drain_pair(pend[2], pend[3], pend[1])
                    avq, box = make_av_groups(c, u, pts)
                    pend = (avq, box, c, u)

            # ---- tail: every A*V group of the last pair contains an
            # i=7 matmul that waits on the final exp, so first burn ALL
            # remaining fill thunks (covering the exp-stream drain), then
            # flush the A*V groups with per-q-tile drain + transpose
            # interleaved so the ctx^T columns start flowing immediately ----
            while work:
                work.popleft()()
            fl = list(pend[0])
            lc, lu, lbox = pend[2], pend[3], pend[1]
            for j in range(4):
                fl[2 * j]()
                fl[2 * j + 1]()
                for par, key in ((0, "e"), (1, "o")):
                    h = 2 * lu + par
                    av = lbox[key]
                    rec = normp.tile([128, 1, 1], f32, tag="rec")
                    nc.vector.reciprocal(rec[:], av[:, j:j + 1, DH:DH + 1])
                    nc.vector.tensor_mul(
                        ctxn_sb[:, j:j + 1, h, :],
                        av[:, j:j + 1, 0:DH],
                        rec[:].to_broadcast([128, 1, DH]),
                    )
                nc.sync.dma_start_transpose(
                    ctxT_sb[:, lu, lc * 512 + j * 128:lc * 512 + (j + 1) * 128],
                    ctxn_sb[:, j, 2 * lu:2 * lu + 2, :]
                    .rearrange("p h e -> p (h e)"),
                )
            engs = [None, nc.sync, nc.scalar]
            for to in range(4, 8):
                for n in range(2):
                    for f in g_out_proj(to, n, engs[(2 * to + n) % 3]):
                        f()

    nc.compile()
    return nc


def _host_prep(x, Wq, bq, Wk, bk, Wv, bv, Wo, bo):
    import ml_dtypes

    bf = ml_dtypes.bfloat16
    x = np.ascontiguousarray(np.asarray(x, dtype=np.float32))
    mats = {
        "wqT": np.ascontiguousarray(np.asarray(Wq, np.float32).T.astype(bf)),
        "wkT": np.ascontiguousarray(np.asarray(Wk, np.float32).T.astype(bf)),
        "wvT": np.ascontiguousarray(np.asarray(Wv, np.float32).T.astype(bf)),
        "woT": np.ascontiguousarray(np.asarray(Wo, np.float32).T.astype(bf)),
        "bqr": np.ascontiguousarray(
            np.asarray(bq, np.float32).reshape(8, 128).T),
        "bkr": np.ascontiguousarray(
            np.asarray(bk, np.float32).reshape(8, 128).T),
        "bvrep": np.ascontiguousarray(
            np.tile(np.asarray(bv, np.float32)[None, :], (128, 1)).astype(bf)),
        "borep": np.ascontiguousarray(
            np.tile(np.asarray(bo, np.float32)[None, :], (128, 1)).astype(bf)),
    }

    in_maps = []
    for core in range(NCORES):
        b, j = core // 4, core % 4
        start = j * BLK
        xkv = np.zeros((NKV, D), np.float32)
        lo = start - W
        if lo < 0:
            xkv[W:] = x[b, start:start + BLK]
        else:
            xkv[:] = x[b, lo:start + BLK]
        vo = np.ones((128, 12), np.float32)
        if j == 0:
            vo[:, 0:4] = 0.0         # halo keys (tokens 0..511) are invalid
        im = dict(mats)
        im["xT"] = np.ascontiguousarray(xkv.T.astype(bf))
        im["vones"] = vo
        in_maps.append(im)
    return in_maps


def kernel(x, Wq, bq, Wk, bk, Wv, bv, Wo, bo):
    from concourse.bass_utils import run_bass_kernel_spmd

    if "nc" not in _CACHE:
        _CACHE["nc"] = _build()
    nc = _CACHE["nc"]

    in_maps = _host_prep(x, Wq, bq, Wk, bk, Wv, bv, Wo, bo)
    res = run_bass_kernel_spmd(nc, in_maps, list(range(NCORES)))

    out = np.empty((B, L, D), np.float32)
    for core in range(NCORES):
        b, j = core // 4, core % 4
        out[b, j * BLK:(j + 1) * BLK] = res.results[core]["out"]
    return out
